# revision 8
# baseline (speedup 1.0000x reference)
"""AdditiveAttention Trainium2 kernel (Bass/Tile), 8-core data-parallel.

Math (per batch b):
    q = queries @ Wq.T              [Q, H]
    k = keys @ Wk.T                 [K, H]
    scores[q,k] = sum_h Wv[h] * tanh(q[q,h] + k[k,h])
    attn = softmax(mask(scores))    positions >= valid_len -> 0 weight
    out = attn @ values             [Q, V]

Algorithm: tanh(x) on |x|<=11.45 approximated by a half-integer sine series
    tanh(x) ~= sum_m c_m sin(w_m x),   w_m = (m - 1/2) * pi / L
Each term factorizes over the q/k split
    sin(w(q+k)) = sin(wq)cos(wk) + cos(wq)sin(wk)
so scores become PE contractions over (h, m, sin/cos).

v2 implementation notes:
  - Both local batches share one packed basis stream
    [h, ht, {sin,cos}, (Qa|Qb|Ka|Kb)]; the Chebyshev ladder
    (s_{m+1} = C s_m - s_{m-1}, C = 2cos(th) = 2-4sin^2(th/2)) runs with a
    column split between DVE and GpSimd (Pool) executing concurrently.
  - Inputs are DMAd in fp32; the fp16 conversion is fused into the
    PSUM->SBUF eviction copies of the PE transposes (ACT), and into the
    Pool copy for values.  C's square comes from ACT (Square shares the
    trig table set with Sin).
  - The repeat loop is unrolled x2 with independent buffer sets and the
    next iteration's DMA loads issued at the tail of each half-body, so
    consecutive iterations software-pipeline.  ACT work is grouped
    [seeds A | seeds B] (trig set) then [exp/copies/out A | B] (exp set):
    one activation-table load per iteration on average.
  - Masking is exact and free: columns k >= valid_len are never computed
    (programs are specialized per (vl_a, vl_b) pair).
"""

import math
import numpy as np

import jax
from jax.sharding import SingleDeviceSharding

import concourse.bass as bass
import concourse.mybir as mybir
import concourse.tile as tile
from concourse import bacc, bass2jax
from concourse.masks import make_identity

B, Q, K, H, V = 16, 128, 512, 256, 256
N_CORES = 8
B_LOC = B // N_CORES  # 2 batches per core
P = 128
HT = H // P   # 2 h-tiles
DT = H // P   # 2 d-tiles (projection contraction)
F32 = mybir.dt.float32
F16 = mybir.dt.float16

# tanh(x) ~= sum_m C_SIN[m] * sin((m+0.5)*pi/L * x); minimax fit |x|<=11.45
L_FIT = 11.6
TH = math.pi / L_FIT
C_SIN = [1.26359946, 0.39701109, 0.21221088, 0.12884517, 0.08154461,
         0.05276574, 0.03415187, 0.02241366, 0.01448774, 0.01666857]
M_HARM = len(C_SIN)

# Fraction of ladder columns executed on GpSimd (Pool) instead of DVE.
POOL_FRAC = 0.38


def _ceil_even(x):
    return (x + 1) // 2 * 2


class Phase:
    """Column layout + buffer-tag suffix for one pipeline phase."""

    def __init__(self, name, vls):
        self.name = name
        self.vls = vls
        self.vle = [_ceil_even(v) for v in vls]
        self.kbn = [(v + P - 1) // P for v in vls]
        # stream column layout: [qa | qb | ka | kb]
        self.kcol = [2 * Q, 2 * Q + self.vle[0]]
        self.NP = 2 * Q + self.vle[0] + self.vle[1]
        self.st = {}


class Emitter:
    def __init__(self, nc, tc, pools, consts, dram):
        self.nc = nc
        self.tc = tc
        self.pools = pools
        self.consts = consts
        self.dram = dram

    # ---------------- loads: fp32 DMAs for the NEXT use of this phase ------
    def loads(self, ph):
        nc = self.nc
        st = ph.st
        stage = self.pools["stage"]
        tg = ph.name
        (queries_d, keys_d, values_d, wq_d, wk_d, out_d) = self.dram

        st["wq32"] = stage.tile([P, HT, H], F32, tag=f"wq32{tg}", name="wq32")
        nc.sync.dma_start(out=st["wq32"],
                          in_=wq_d.rearrange("(t p) d -> p t d", p=P))
        st["wk32"] = stage.tile([P, HT, H], F32, tag=f"wk32{tg}", name="wk32")
        nc.sync.dma_start(out=st["wk32"],
                          in_=wk_d.rearrange("(t p) d -> p t d", p=P))
        st["q32"] = stage.tile([P, B_LOC, H], F32, tag=f"q32{tg}", name="q32")
        nc.sync.dma_start(out=st["q32"],
                          in_=queries_d.rearrange("b q d -> q b d"))
        st["k32"] = []
        st["v32"] = []
        for b in range(B_LOC):
            kb = ph.kbn[b]
            kt_ = stage.tile([P, kb, H], F32, tag=f"k32{tg}{b}", name="k32")
            nc.sync.dma_start(
                out=kt_,
                in_=keys_d[b, :kb * P].rearrange("(kt p) d -> p kt d", p=P))
            st["k32"].append(kt_)
            vt = stage.tile([P, kb, V], F32, tag=f"v32{tg}{b}", name="v32")
            nc.sync.dma_start(
                out=vt,
                in_=values_d[b, :kb * P].rearrange("(kt p) d -> p kt d", p=P))
            st["v32"].append(vt)

    # ---------------- part 1: transposes, projections, seeds ---------------
    def part1(self, ph):
        nc = self.nc
        st = ph.st
        tg = ph.name
        vls, vle, kbn, NP = ph.vls, ph.vle, ph.kbn, ph.NP
        stage = self.pools["stage"]
        xpool = self.pools["xpool"]
        ps_pq = self.pools["ps_pq"]
        ps_pk = self.pools["ps_pk"]
        ps_tr32 = self.pools["ps_tr32"]
        identf32 = self.consts["identf32"]
        bias_z = self.consts["bias_z"]
        bias_ph = self.consts["bias_ph"]

        tr_state = {"tile": None, "j": 4}

        def tr_slot():
            if tr_state["j"] == 4:
                tr_state["tile"] = ps_tr32.tile([P, 4, P], F32,
                                                tag="tr32", name="tr32")
                tr_state["j"] = 0
            j = tr_state["j"]
            tr_state["j"] += 1
            return tr_state["tile"][:, j, :]

        # values: convert on Pool, append ones column
        st["vo"] = []
        for b in range(B_LOC):
            kb = kbn[b]
            vo = stage.tile([P, kb, V + 1], F16, tag=f"vo{tg}{b}", name="vo")
            nc.gpsimd.tensor_copy(out=vo[:, :, :V], in_=st["v32"][b])
            nc.gpsimd.memset(vo[:, :, V:V + 1], 1.0)
            st["vo"].append(vo)

        # fp32 PE transposes; the evict copy converts to fp16 on ACT
        wqT = stage.tile([P, DT, H], F16, tag=f"wqT{tg}")  # [d_in, dt, h]
        wkT = stage.tile([P, DT, H], F16, tag=f"wkT{tg}")
        for (w32, w_T) in ((st["wq32"], wqT), (st["wk32"], wkT)):
            for ht in range(HT):
                for dt in range(DT):
                    ps = tr_slot()
                    nc.tensor.transpose(ps, w32[:, ht, dt * P:(dt + 1) * P],
                                        identf32)
                    nc.scalar.copy(out=w_T[:, dt, ht * P:(ht + 1) * P], in_=ps)
        qT = stage.tile([P, DT, 2 * Q], F16, tag=f"qT{tg}")  # [d, dt, qa|qb]
        for b in range(B_LOC):
            for dt in range(DT):
                ps = tr_slot()
                nc.tensor.transpose(ps, st["q32"][:, b, dt * P:(dt + 1) * P],
                                    identf32)
                nc.scalar.copy(out=qT[:, dt, b * Q:(b + 1) * Q], in_=ps)
        kT = []
        for b in range(B_LOC):
            kb = kbn[b]
            kTb = stage.tile([P, DT, kb * P], F16, tag=f"kT{tg}{b}", name="kTb")
            for kt_i in range(kb):
                for dt in range(DT):
                    ps = tr_slot()
                    nc.tensor.transpose(
                        ps, st["k32"][b][:, kt_i, dt * P:(dt + 1) * P],
                        identf32)
                    nc.scalar.copy(out=kTb[:, dt, kt_i * P:(kt_i + 1) * P],
                                   in_=ps)
            kT.append(kTb)

        # projections into PSUM; ACT Sin seeds straight from PSUM
        X1 = xpool.tile([P, HT, 2, NP], F16, tag=f"X1{tg}")
        st["X1"] = X1
        for ht in range(HT):
            pq = ps_pq.tile([P, 2 * Q], F32, tag="pq", name="pq")
            for dt in range(DT):
                nc.tensor.matmul(pq, wqT[:, dt, ht * P:(ht + 1) * P],
                                 qT[:, dt, :],
                                 start=(dt == 0), stop=(dt == DT - 1))
            for (side, bias) in ((0, bias_z), (1, bias_ph)):
                nc.scalar.activation(out=X1[:, ht, side, 0:2 * Q], in_=pq,
                                     func=mybir.ActivationFunctionType.Sin,
                                     bias=bias, scale=TH / 2)
            for b in range(B_LOC):
                ve = vle[b]
                pk = ps_pk.tile([P, 512], F32, tag="pk", name="pk")
                for dt in range(DT):
                    nc.tensor.matmul(pk[:, :ve],
                                     wkT[:, dt, ht * P:(ht + 1) * P],
                                     kT[b][:, dt, :ve],
                                     start=(dt == 0), stop=(dt == DT - 1))
                c0 = ph.kcol[b]
                for (side, bias) in ((0, bias_z), (1, bias_ph)):
                    nc.scalar.activation(
                        out=X1[:, ht, side, c0:c0 + ve], in_=pk[:, :ve],
                        func=mybir.ActivationFunctionType.Sin,
                        bias=bias, scale=TH / 2)
        # u = sin(th/2)^2 for the C stream (Square lives in the trig set)
        usq = stage.tile([P, HT, NP], F16, tag=f"usq{tg}")
        nc.scalar.activation(out=usq, in_=X1[:, :, 0, :],
                             func=mybir.ActivationFunctionType.Square,
                             bias=bias_z)
        st["usq"] = usq

    # ---------------- part 2: ladder, scores, softmax, AV ------------------
    def part2(self, ph):
        nc = self.nc
        st = ph.st
        tg = ph.name
        vls, vle, kbn, NP = ph.vls, ph.vle, ph.kbn, ph.NP
        stage = self.pools["stage"]
        xpool = self.pools["xpool"]
        bpool = self.pools["bpool"]
        btpool = self.pools["btpool"]
        sqpool = self.pools["sqpool"]
        ps_sc = self.pools["ps_sc"]
        ps_tr16 = self.pools["ps_tr16"]
        ps_po = self.pools["ps_po"]
        identf = self.consts["identf"]
        wv_sb = self.consts["wv_sb"]
        bias_z = self.consts["bias_z"]
        X1 = st["X1"]

        split = min(NP, _ceil_even(int(NP * (1.0 - POOL_FRAC))))

        def split_tt(dst, a_, b_, op):
            nc.vector.tensor_tensor(dst[:, :, :, 0:split],
                                    a_[:, :, :, 0:split],
                                    b_[:, :, :, 0:split], op)
            if split < NP:
                nc.gpsimd.tensor_tensor(dst[:, :, :, split:NP],
                                        a_[:, :, :, split:NP],
                                        b_[:, :, :, split:NP], op)

        # C streams: C = 2 - 4u, duplicated over the {sin,cos} axis
        C2 = xpool.tile([P, HT, 2, NP], F16, tag=f"C2{tg}")
        nc.vector.tensor_scalar(out=C2[:, :, 0, :], in0=st["usq"],
                                scalar1=-4.0, scalar2=2.0,
                                op0=mybir.AluOpType.mult,
                                op1=mybir.AluOpType.add)
        nc.vector.tensor_copy(out=C2[:, :, 1, :], in_=C2[:, :, 0, :])
        Cp1 = stage.tile([P, HT, NP], F16, tag=f"Cp{tg}")
        nc.vector.tensor_scalar_add(Cp1, C2[:, :, 0, :], 1.0)
        Cm1 = stage.tile([P, HT, NP], F16, tag=f"Cm{tg}")
        nc.vector.tensor_scalar_add(Cm1, C2[:, :, 0, :], -1.0)

        sc_ps = [ps_sc.tile([P, 512], F32, tag=f"sc{b}", name="sc")
                 for b in range(B_LOC)]

        def score_mms(m, basis):
            if m == 1:
                sq = sqpool.tile([P, HT, 2, 2 * Q], F16, tag=f"sq{tg}",
                                 name="sq1")
                for ht in range(HT):
                    nc.vector.tensor_scalar(
                        out=sq[:, ht], in0=basis[:, ht, :, 0:2 * Q],
                        scalar1=wv_sb[:, ht:ht + 1], scalar2=float(C_SIN[0]),
                        op0=mybir.AluOpType.mult, op1=mybir.AluOpType.mult)
            else:
                sq = sqpool.tile([P, HT, 2, 2 * Q], F16, tag=f"sq{tg}",
                                 name="sqm")
                nc.vector.tensor_scalar_mul(sq, basis[:, :, :, 0:2 * Q],
                                            float(C_SIN[m - 1]))
            for b in range(B_LOC):
                vl = vls[b]
                c0 = ph.kcol[b]
                for ht in range(HT):
                    first = (ht == 0 and m == 1)
                    last = (ht == HT - 1 and m == M_HARM)
                    nc.tensor.matmul(sc_ps[b][:, :vl],
                                     sq[:, ht, 0, b * Q:(b + 1) * Q],
                                     basis[:, ht, 1, c0:c0 + vl],
                                     start=first, stop=False)
                    nc.tensor.matmul(sc_ps[b][:, :vl],
                                     sq[:, ht, 1, b * Q:(b + 1) * Q],
                                     basis[:, ht, 0, c0:c0 + vl],
                                     start=False, stop=last)

        score_mms(1, X1)
        # fold Wv into q-cols of the seed stream; later harmonics inherit
        for ht in range(HT):
            nc.vector.tensor_scalar_mul(X1[:, ht, :, 0:2 * Q],
                                        X1[:, ht, :, 0:2 * Q],
                                        wv_sb[:, ht:ht + 1])
        b2 = bpool.tile([P, HT, 2, NP], F16, tag=f"b{tg}")
        nc.vector.tensor_tensor(b2[:, :, 0, :], Cp1, X1[:, :, 0, :],
                                mybir.AluOpType.mult)
        nc.vector.tensor_tensor(b2[:, :, 1, :], Cm1, X1[:, :, 1, :],
                                mybir.AluOpType.mult)
        score_mms(2, b2)
        prev2, prev = X1, b2
        for m in range(3, M_HARM + 1):
            t = btpool.tile([P, HT, 2, NP], F16, tag=f"bt{tg}", name="bt")
            split_tt(t, C2, prev, mybir.AluOpType.mult)
            bm = bpool.tile([P, HT, 2, NP], F16, tag=f"b{tg}")
            split_tt(bm, t, prev2, mybir.AluOpType.subtract)
            score_mms(m, bm)
            prev2, prev = prev, bm

        # epilogue: exp (from score PSUM), transpose, AV, normalize
        tr_state = {"tile": None, "j": 4}

        def tr_slot():
            if tr_state["j"] == 4:
                tr_state["tile"] = ps_tr16.tile([P, 4, P], F16,
                                                tag="tr16", name="tr16")
                tr_state["j"] = 0
            j = tr_state["j"]
            tr_state["j"] += 1
            return tr_state["tile"][:, j, :]

        for b in range(B_LOC):
            vl = vls[b]
            kb = kbn[b]
            e = stage.tile([P, K], F16, tag=f"e{tg}{b}")
            nc.scalar.activation(out=e[:, :vl], in_=sc_ps[b][:, :vl],
                                 func=mybir.ActivationFunctionType.Exp,
                                 bias=bias_z)
            po = ps_po.tile([P, V + 1], F32, tag="po", name="po")
            for kt_i in range(kb):
                cols = min(P, vl - kt_i * P)
                ps = tr_slot()
                nc.tensor.transpose(ps[:cols, :],
                                    e[:, kt_i * P:kt_i * P + cols], identf)
                eT = stage.tile([P, Q], F16, tag=f"eT{tg}")
                nc.scalar.copy(out=eT[:cols, :], in_=ps[:cols, :])
                nc.tensor.matmul(po, eT[:cols, :], st["vo"][b][:cols, kt_i, :],
                                 start=(kt_i == 0), stop=(kt_i == kb - 1))
            r = stage.tile([P, 1], F32, tag=f"recip{tg}{b}")
            nc.vector.reciprocal(out=r, in_=po[:, V:V + 1])
            ot = stage.tile([P, V], F32, tag=f"ot{tg}{b}")
            nc.scalar.activation(out=ot, in_=po[:, :V],
                                 func=mybir.ActivationFunctionType.Copy,
                                 scale=r)
            nc.sync.dma_start(out=self.dram[5][b], in_=ot)


def build_nc(vls, repeat=1):
    """vls: (vl_a, vl_b) exact K-extents for the two local batches."""
    from contextlib import ExitStack
    nc = bacc.Bacc("TRN2", target_bir_lowering=False, debug=False,
                   num_devices=N_CORES, enable_partition_id=False)
    queries_d = nc.dram_tensor("queries", [B_LOC, Q, H], F32, kind="ExternalInput").ap()
    keys_d = nc.dram_tensor("keys", [B_LOC, K, H], F32, kind="ExternalInput").ap()
    values_d = nc.dram_tensor("values", [B_LOC, K, V], F32, kind="ExternalInput").ap()
    wq_d = nc.dram_tensor("Wq", [H, H], F32, kind="ExternalInput").ap()
    wk_d = nc.dram_tensor("Wk", [H, H], F32, kind="ExternalInput").ap()
    wv_d = nc.dram_tensor("Wv", [H], F32, kind="ExternalInput").ap()
    out_d = nc.dram_tensor("out", [B_LOC, Q, V], F32, kind="ExternalOutput").ap()
    dram = (queries_d, keys_d, values_d, wq_d, wk_d, out_d)

    with tile.TileContext(nc) as tc, ExitStack() as ctx:
        const = ctx.enter_context(tc.tile_pool(name="const", bufs=1))
        identf = const.tile([P, P], F16)
        make_identity(nc, identf)
        identf32 = const.tile([P, P], F32)
        make_identity(nc, identf32)
        wv_sb = const.tile([P, HT], F32)
        nc.sync.dma_start(out=wv_sb, in_=wv_d.rearrange("(t p) -> p t", p=P))
        bias_z = const.tile([P, 1], F32)
        nc.gpsimd.memset(bias_z, 0.0)
        bias_ph = const.tile([P, 1], F32)
        nc.gpsimd.memset(bias_ph, math.pi / 2)
        consts = {"identf": identf, "identf32": identf32, "wv_sb": wv_sb,
                  "bias_z": bias_z, "bias_ph": bias_ph}

        pools = {
            "stage": ctx.enter_context(tc.tile_pool(name="stage", bufs=1)),
            "xpool": ctx.enter_context(tc.tile_pool(name="xpool", bufs=1)),
            "bpool": ctx.enter_context(tc.tile_pool(name="bpool", bufs=3)),
            "btpool": ctx.enter_context(tc.tile_pool(name="btpool", bufs=2)),
            "sqpool": ctx.enter_context(tc.tile_pool(name="sq", bufs=2)),
            "ps_pq": ctx.enter_context(
                tc.tile_pool(name="pspq", bufs=1, space="PSUM")),
            "ps_pk": ctx.enter_context(
                tc.tile_pool(name="pspk", bufs=2, space="PSUM")),
            "ps_sc": ctx.enter_context(
                tc.tile_pool(name="pssc", bufs=1, space="PSUM")),
            "ps_tr32": ctx.enter_context(
                tc.tile_pool(name="pst32", bufs=1, space="PSUM")),
            "ps_tr16": ctx.enter_context(
                tc.tile_pool(name="pst16", bufs=1, space="PSUM")),
            "ps_po": ctx.enter_context(
                tc.tile_pool(name="pspo", bufs=1, space="PSUM")),
        }
        em = Emitter(nc, tc, pools, consts, dram)
        phA = Phase("A", vls)
        phB = Phase("B", vls)

        if repeat <= 6:
            phs = [phA, phB]
            for i in range(0, repeat, 2):
                pair = phs if repeat - i >= 2 else phs[:1]
                for p in pair:
                    em.loads(p)
                for p in pair:
                    em.part1(p)
                for p in pair:
                    em.part2(p)
        else:
            n2 = repeat // 2
            rem = repeat - 2 * n2
            with tc.For_i(0, n2, 1):
                em.loads(phA)
                em.loads(phB)
                em.part1(phA)
                em.part1(phB)
                em.part2(phA)
                em.part2(phB)
            if rem:
                em.loads(phA)
                em.part1(phA)
                em.part2(phA)
    nc.compile()
    return nc


def _make_single_core_runner(nc, device):
    """jit the program once for one device; reusable across calls."""
    bass2jax.install_neuronx_cc_hook()
    assert nc.partition_id_tensor is None
    in_names, out_names, out_avals, zero_shapes = [], [], [], []
    for alloc in nc.m.functions[0].allocations:
        if not isinstance(alloc, mybir.MemoryLocationSet):
            continue
        name = alloc.memorylocations[0].name
        if alloc.kind == "ExternalInput":
            in_names.append(name)
        elif alloc.kind == "ExternalOutput":
            shape = tuple(alloc.tensor_shape)
            npdt = np.dtype(mybir.dt.np(alloc.dtype))
            out_names.append(name)
            out_avals.append(jax.core.ShapedArray(shape, npdt))
            zero_shapes.append((shape, npdt))
    n_params = len(in_names)
    n_outs = len(out_avals)
    in_names_all = list(in_names) + list(out_names)

    def _body(*args):
        outs = bass2jax._bass_exec_p.bind(
            *args,
            out_avals=tuple(out_avals),
            in_names=tuple(in_names_all),
            out_names=tuple(out_names),
            lowering_input_output_aliases=(),
            sim_require_finite=True,
            sim_require_nnan=True,
            nc=nc,
        )
        return tuple(outs)

    fn = jax.jit(_body, donate_argnums=tuple(range(n_params, n_params + n_outs)),
                 keep_unused=True)
    sharding = SingleDeviceSharding(device)
    dev_in_cache = {}

    def launch(in_map):
        key = id(in_map)
        if key not in dev_in_cache:
            dev_in_cache.clear()
            dev_in_cache[key] = [
                jax.device_put(np.asarray(in_map[name]), sharding)
                for name in in_names
            ]
        args = list(dev_in_cache[key])
        args += [jax.device_put(np.zeros(s, d), sharding) for (s, d) in zero_shapes]
        outs = fn(*args)
        return dict(zip(out_names, outs))

    return launch


_NCS = {}       # (vls, repeat) -> compiled nc
_LAUNCH = {}    # (vls, repeat, core) -> launch fn


def _get_launch(vls, repeat, core):
    key = (vls, repeat, core)
    if key not in _LAUNCH:
        nckey = (vls, repeat)
        if nckey not in _NCS:
            _NCS[nckey] = build_nc(vls, repeat)
        _LAUNCH[key] = _make_single_core_runner(_NCS[nckey], jax.devices()[core])
    return _LAUNCH[key]


def plan_assignment(valid_lens):
    """Pair batches to balance per-core work; returns (perm, vls_per_core).

    perm[2c], perm[2c+1] are the global batch indices handled by core c.
    """
    vle = [min(K, int(v)) for v in valid_lens]
    order = sorted(range(B), key=lambda i: -vle[i])
    perm, vls_per_core = [], []
    for c in range(N_CORES):
        a, b_ = order[c], order[2 * N_CORES - 1 - c]
        perm += [a, b_]
        vls_per_core.append((vle[a], vle[b_]))
    return perm, vls_per_core


def run_cores(in_maps, vls_per_core, repeat=1, fetch=True):
    """Launch all 8 per-core programs concurrently; returns per-core out dicts."""
    outs = [
        _get_launch(vls_per_core[c], repeat, c)(in_maps[c]) for c in range(N_CORES)
    ]
    jax.block_until_ready([list(o.values()) for o in outs])
    if not fetch:
        return None
    return [{k: np.asarray(v) for k, v in o.items()} for o in outs]


def make_in_maps(queries, keys, values, Wq, Wk, Wv, valid_lens, perm):
    queries = np.asarray(queries, np.float32)
    keys = np.asarray(keys, np.float32)
    values = np.asarray(values, np.float32)
    Wq = np.asarray(Wq, np.float32)
    Wk = np.asarray(Wk, np.float32)
    Wv = np.asarray(Wv, np.float32)
    in_maps = []
    for c in range(N_CORES):
        ix = [perm[2 * c], perm[2 * c + 1]]
        in_maps.append({
            "queries": queries[ix], "keys": keys[ix], "values": values[ix],
            "Wq": Wq, "Wk": Wk, "Wv": Wv,
        })
    return in_maps


def kernel(queries, keys, values, Wq, Wk, Wv, valid_lens):
    perm, vls_per_core = plan_assignment(valid_lens)
    in_maps = make_in_maps(queries, keys, values, Wq, Wk, Wv, valid_lens, perm)
    res = run_cores(in_maps, vls_per_core)
    out = np.empty((B, Q, V), np.float32)
    for c in range(N_CORES):
        out[perm[2 * c]] = res[c]["out"][0]
        out[perm[2 * c + 1]] = res[c]["out"][1]
    return out


# revision 9
# speedup vs baseline: 1.1637x; 1.1637x over previous
"""AdditiveAttention Trainium2 kernel (Bass/Tile), 8-core data-parallel.

Math (per batch b):
    q = queries @ Wq.T              [Q, H]
    k = keys @ Wk.T                 [K, H]
    scores[q,k] = sum_h Wv[h] * tanh(q[q,h] + k[k,h])
    attn = softmax(mask(scores))    positions >= valid_len -> 0 weight
    out = attn @ values             [Q, V]

Algorithm: tanh(x) on |x|<=11.45 approximated by a half-integer sine series
    tanh(x) ~= sum_m c_m sin(w_m x),   w_m = (m - 1/2) * pi / L
Each term factorizes over the q/k split
    sin(w(q+k)) = sin(wq)cos(wk) + cos(wq)sin(wk)
so scores become PE contractions over (h, m, sin/cos).

v2 implementation notes:
  - Both local batches share one packed basis stream
    [h, ht, {sin,cos}, (Qa|Qb|Ka|Kb)]; the Chebyshev ladder
    (s_{m+1} = C s_m - s_{m-1}, C = 2cos(th) = 2-4sin^2(th/2)) runs with a
    column split between DVE and GpSimd (Pool) executing concurrently.
  - Inputs are DMAd in fp32; the fp16 conversion is fused into the
    PSUM->SBUF eviction copies of the PE transposes (ACT), and into the
    Pool copy for values.  C's square comes from ACT (Square shares the
    trig table set with Sin).
  - The repeat loop is unrolled x2 with independent buffer sets and the
    next iteration's DMA loads issued at the tail of each half-body, so
    consecutive iterations software-pipeline.  ACT work is grouped
    [seeds A | seeds B] (trig set) then [exp/copies/out A | B] (exp set):
    one activation-table load per iteration on average.
  - Masking is exact and free: columns k >= valid_len are never computed
    (programs are specialized per (vl_a, vl_b) pair).
"""

import math
import numpy as np

import jax
from jax.sharding import SingleDeviceSharding

import concourse.bass as bass
import concourse.mybir as mybir
import concourse.tile as tile
from concourse import bacc, bass2jax
from concourse.masks import make_identity

B, Q, K, H, V = 16, 128, 512, 256, 256
N_CORES = 8
B_LOC = B // N_CORES  # 2 batches per core
P = 128
HT = H // P   # 2 h-tiles
DT = H // P   # 2 d-tiles (projection contraction)
F32 = mybir.dt.float32
F16 = mybir.dt.float16

# tanh(x) ~= sum_m C_SIN[m] * sin((m+0.5)*pi/L * x); minimax fit |x|<=11.45
L_FIT = 11.6
TH = math.pi / L_FIT
C_SIN = [1.26359946, 0.39701109, 0.21221088, 0.12884517, 0.08154461,
         0.05276574, 0.03415187, 0.02241366, 0.01448774, 0.01666857]
M_HARM = len(C_SIN)

# Fraction of ladder columns executed on GpSimd (Pool) instead of DVE.
POOL_FRAC = 0.0


def _ceil_even(x):
    return (x + 1) // 2 * 2


class Phase:
    """Column layout + buffer-tag suffix for one pipeline phase."""

    def __init__(self, name, vls):
        self.name = name
        self.vls = vls
        self.vle = [_ceil_even(v) for v in vls]
        self.kbn = [(v + P - 1) // P for v in vls]
        # stream column layout: [qa | qb | ka | kb]
        self.kcol = [2 * Q, 2 * Q + self.vle[0]]
        self.NP = 2 * Q + self.vle[0] + self.vle[1]
        self.st = {}


class Emitter:
    def __init__(self, nc, tc, pools, consts, dram):
        self.nc = nc
        self.tc = tc
        self.pools = pools
        self.consts = consts
        self.dram = dram

    # ---------------- loads: fp32 DMAs for the NEXT use of this phase ------
    def loads(self, ph):
        nc = self.nc
        st = ph.st
        stage = self.pools["stage"]
        tg = ph.name
        (queries_d, keys_d, values_d, wq_d, wk_d, out_d) = self.dram

        st["wq32"] = stage.tile([P, HT, H], F32, tag=f"wq32{tg}", name="wq32")
        nc.sync.dma_start(out=st["wq32"],
                          in_=wq_d.rearrange("(t p) d -> p t d", p=P))
        st["wk32"] = stage.tile([P, HT, H], F32, tag=f"wk32{tg}", name="wk32")
        nc.sync.dma_start(out=st["wk32"],
                          in_=wk_d.rearrange("(t p) d -> p t d", p=P))
        st["q32"] = stage.tile([P, B_LOC, H], F32, tag=f"q32{tg}", name="q32")
        nc.sync.dma_start(out=st["q32"],
                          in_=queries_d.rearrange("b q d -> q b d"))
        st["k32"] = []
        st["v32"] = []
        for b in range(B_LOC):
            kb = ph.kbn[b]
            kt_ = stage.tile([P, kb, H], F32, tag=f"k32{tg}{b}", name="k32")
            nc.sync.dma_start(
                out=kt_,
                in_=keys_d[b, :kb * P].rearrange("(kt p) d -> p kt d", p=P))
            st["k32"].append(kt_)
            vt = stage.tile([P, kb, V], F32, tag=f"v32{tg}{b}", name="v32")
            nc.sync.dma_start(
                out=vt,
                in_=values_d[b, :kb * P].rearrange("(kt p) d -> p kt d", p=P))
            st["v32"].append(vt)

    # ---------------- part 1: transposes, projections, seeds ---------------
    def part1(self, ph):
        nc = self.nc
        st = ph.st
        tg = ph.name
        vls, vle, kbn, NP = ph.vls, ph.vle, ph.kbn, ph.NP
        stage = self.pools["stage"]
        xpool = self.pools["xpool"]
        ps_pq = self.pools["ps_pq"]
        ps_pk = self.pools["ps_pk"]
        ps_tr32 = self.pools["ps_tr32"]
        identf32 = self.consts["identf32"]
        bias_z = self.consts["bias_z"]
        bias_ph = self.consts["bias_ph"]

        tr_state = {"tile": None, "j": 4}

        def tr_slot():
            if tr_state["j"] == 4:
                tr_state["tile"] = ps_tr32.tile([P, 4, P], F32,
                                                tag="tr32", name="tr32")
                tr_state["j"] = 0
            j = tr_state["j"]
            tr_state["j"] += 1
            return tr_state["tile"][:, j, :]

        # values: convert on Pool, append ones column
        st["vo"] = []
        for b in range(B_LOC):
            kb = kbn[b]
            vo = stage.tile([P, kb, V + 1], F16, tag=f"vo{tg}{b}", name="vo")
            nc.gpsimd.tensor_copy(out=vo[:, :, :V], in_=st["v32"][b])
            nc.gpsimd.memset(vo[:, :, V:V + 1], 1.0)
            st["vo"].append(vo)

        # fp32 PE transposes; the evict copy converts to fp16 on ACT
        wqT = stage.tile([P, DT, H], F16, tag=f"wqT{tg}")  # [d_in, dt, h]
        wkT = stage.tile([P, DT, H], F16, tag=f"wkT{tg}")
        for (w32, w_T) in ((st["wq32"], wqT), (st["wk32"], wkT)):
            for ht in range(HT):
                for dt in range(DT):
                    ps = tr_slot()
                    nc.tensor.transpose(ps, w32[:, ht, dt * P:(dt + 1) * P],
                                        identf32)
                    nc.scalar.copy(out=w_T[:, dt, ht * P:(ht + 1) * P], in_=ps)
        qT = stage.tile([P, DT, 2 * Q], F16, tag=f"qT{tg}")  # [d, dt, qa|qb]
        for b in range(B_LOC):
            for dt in range(DT):
                ps = tr_slot()
                nc.tensor.transpose(ps, st["q32"][:, b, dt * P:(dt + 1) * P],
                                    identf32)
                nc.scalar.copy(out=qT[:, dt, b * Q:(b + 1) * Q], in_=ps)
        kT = []
        for b in range(B_LOC):
            kb = kbn[b]
            kTb = stage.tile([P, DT, kb * P], F16, tag=f"kT{tg}{b}", name="kTb")
            for kt_i in range(kb):
                for dt in range(DT):
                    ps = tr_slot()
                    nc.tensor.transpose(
                        ps, st["k32"][b][:, kt_i, dt * P:(dt + 1) * P],
                        identf32)
                    nc.scalar.copy(out=kTb[:, dt, kt_i * P:(kt_i + 1) * P],
                                   in_=ps)
            kT.append(kTb)

        # projections into PSUM; ACT Sin seeds straight from PSUM
        X1 = xpool.tile([P, HT, 2, NP], F16, tag=f"X1{tg}")
        st["X1"] = X1
        for ht in range(HT):
            pq = ps_pq.tile([P, 2 * Q], F32, tag="pq", name="pq")
            for dt in range(DT):
                nc.tensor.matmul(pq, wqT[:, dt, ht * P:(ht + 1) * P],
                                 qT[:, dt, :],
                                 start=(dt == 0), stop=(dt == DT - 1))
            for (side, bias) in ((0, bias_z), (1, bias_ph)):
                nc.scalar.activation(out=X1[:, ht, side, 0:2 * Q], in_=pq,
                                     func=mybir.ActivationFunctionType.Sin,
                                     bias=bias, scale=TH / 2)
            for b in range(B_LOC):
                ve = vle[b]
                pk = ps_pk.tile([P, 512], F32, tag="pk", name="pk")
                for dt in range(DT):
                    nc.tensor.matmul(pk[:, :ve],
                                     wkT[:, dt, ht * P:(ht + 1) * P],
                                     kT[b][:, dt, :ve],
                                     start=(dt == 0), stop=(dt == DT - 1))
                c0 = ph.kcol[b]
                for (side, bias) in ((0, bias_z), (1, bias_ph)):
                    nc.scalar.activation(
                        out=X1[:, ht, side, c0:c0 + ve], in_=pk[:, :ve],
                        func=mybir.ActivationFunctionType.Sin,
                        bias=bias, scale=TH / 2)
        # u = sin(th/2)^2 for the C stream (Square lives in the trig set)
        usq = stage.tile([P, HT, NP], F16, tag=f"usq{tg}")
        nc.scalar.activation(out=usq, in_=X1[:, :, 0, :],
                             func=mybir.ActivationFunctionType.Square,
                             bias=bias_z)
        st["usq"] = usq

    # ---------------- part 2: ladder, scores, softmax, AV ------------------
    def part2(self, ph):
        nc = self.nc
        st = ph.st
        tg = ph.name
        vls, vle, kbn, NP = ph.vls, ph.vle, ph.kbn, ph.NP
        stage = self.pools["stage"]
        xpool = self.pools["xpool"]
        bpool = self.pools["bpool"]
        btpool = self.pools["btpool"]
        sqpool = self.pools["sqpool"]
        ps_sc = self.pools["ps_sc"]
        ps_tr16 = self.pools["ps_tr16"]
        ps_po = self.pools["ps_po"]
        identf = self.consts["identf"]
        wv_sb = self.consts["wv_sb"]
        bias_z = self.consts["bias_z"]
        X1 = st["X1"]

        split = min(NP, _ceil_even(int(NP * (1.0 - POOL_FRAC))))

        def split_tt(dst, a_, b_, op):
            nc.vector.tensor_tensor(dst[:, :, :, 0:split],
                                    a_[:, :, :, 0:split],
                                    b_[:, :, :, 0:split], op)
            if split < NP:
                nc.gpsimd.tensor_tensor(dst[:, :, :, split:NP],
                                        a_[:, :, :, split:NP],
                                        b_[:, :, :, split:NP], op)

        # C streams: C = 2 - 4u, duplicated over the {sin,cos} axis
        C2 = xpool.tile([P, HT, 2, NP], F16, tag=f"C2{tg}")
        nc.vector.tensor_scalar(out=C2[:, :, 0, :], in0=st["usq"],
                                scalar1=-4.0, scalar2=2.0,
                                op0=mybir.AluOpType.mult,
                                op1=mybir.AluOpType.add)
        nc.vector.tensor_copy(out=C2[:, :, 1, :], in_=C2[:, :, 0, :])
        Cp1 = stage.tile([P, HT, NP], F16, tag=f"Cp{tg}")
        nc.vector.tensor_scalar_add(Cp1, C2[:, :, 0, :], 1.0)
        Cm1 = stage.tile([P, HT, NP], F16, tag=f"Cm{tg}")
        nc.vector.tensor_scalar_add(Cm1, C2[:, :, 0, :], -1.0)

        sc_ps = [ps_sc.tile([P, 512], F32, tag=f"sc{b}", name="sc")
                 for b in range(B_LOC)]

        def score_mms(m, basis):
            if m == 1:
                sq = sqpool.tile([P, HT, 2, 2 * Q], F16, tag=f"sq{tg}",
                                 name="sq1")
                for ht in range(HT):
                    nc.vector.tensor_scalar(
                        out=sq[:, ht], in0=basis[:, ht, :, 0:2 * Q],
                        scalar1=wv_sb[:, ht:ht + 1], scalar2=float(C_SIN[0]),
                        op0=mybir.AluOpType.mult, op1=mybir.AluOpType.mult)
            else:
                sq = sqpool.tile([P, HT, 2, 2 * Q], F16, tag=f"sq{tg}",
                                 name="sqm")
                nc.vector.tensor_scalar_mul(sq, basis[:, :, :, 0:2 * Q],
                                            float(C_SIN[m - 1]))
            for b in range(B_LOC):
                vl = vls[b]
                c0 = ph.kcol[b]
                for ht in range(HT):
                    first = (ht == 0 and m == 1)
                    last = (ht == HT - 1 and m == M_HARM)
                    nc.tensor.matmul(sc_ps[b][:, :vl],
                                     sq[:, ht, 0, b * Q:(b + 1) * Q],
                                     basis[:, ht, 1, c0:c0 + vl],
                                     start=first, stop=False)
                    nc.tensor.matmul(sc_ps[b][:, :vl],
                                     sq[:, ht, 1, b * Q:(b + 1) * Q],
                                     basis[:, ht, 0, c0:c0 + vl],
                                     start=False, stop=last)

        score_mms(1, X1)
        # fold Wv into q-cols of the seed stream; later harmonics inherit
        for ht in range(HT):
            nc.vector.tensor_scalar_mul(X1[:, ht, :, 0:2 * Q],
                                        X1[:, ht, :, 0:2 * Q],
                                        wv_sb[:, ht:ht + 1])
        b2 = bpool.tile([P, HT, 2, NP], F16, tag=f"b{tg}")
        nc.vector.tensor_tensor(b2[:, :, 0, :], Cp1, X1[:, :, 0, :],
                                mybir.AluOpType.mult)
        nc.vector.tensor_tensor(b2[:, :, 1, :], Cm1, X1[:, :, 1, :],
                                mybir.AluOpType.mult)
        score_mms(2, b2)
        prev2, prev = X1, b2
        for m in range(3, M_HARM + 1):
            t = btpool.tile([P, HT, 2, NP], F16, tag=f"bt{tg}", name="bt")
            split_tt(t, C2, prev, mybir.AluOpType.mult)
            bm = bpool.tile([P, HT, 2, NP], F16, tag=f"b{tg}")
            split_tt(bm, t, prev2, mybir.AluOpType.subtract)
            score_mms(m, bm)
            prev2, prev = prev, bm

        # epilogue: exp (from score PSUM), transpose, AV, normalize
        tr_state = {"tile": None, "j": 4}

        def tr_slot():
            if tr_state["j"] == 4:
                tr_state["tile"] = ps_tr16.tile([P, 4, P], F16,
                                                tag="tr16", name="tr16")
                tr_state["j"] = 0
            j = tr_state["j"]
            tr_state["j"] += 1
            return tr_state["tile"][:, j, :]

        for b in range(B_LOC):
            vl = vls[b]
            kb = kbn[b]
            e = stage.tile([P, K], F16, tag=f"e{tg}{b}")
            nc.scalar.activation(out=e[:, :vl], in_=sc_ps[b][:, :vl],
                                 func=mybir.ActivationFunctionType.Exp,
                                 bias=bias_z)
            po = ps_po.tile([P, V + 1], F32, tag="po", name="po")
            for kt_i in range(kb):
                cols = min(P, vl - kt_i * P)
                ps = tr_slot()
                nc.tensor.transpose(ps[:cols, :],
                                    e[:, kt_i * P:kt_i * P + cols], identf)
                eT = stage.tile([P, Q], F16, tag=f"eT{tg}")
                nc.scalar.copy(out=eT[:cols, :], in_=ps[:cols, :])
                nc.tensor.matmul(po, eT[:cols, :], st["vo"][b][:cols, kt_i, :],
                                 start=(kt_i == 0), stop=(kt_i == kb - 1))
            r = stage.tile([P, 1], F32, tag=f"recip{tg}{b}")
            nc.vector.reciprocal(out=r, in_=po[:, V:V + 1])
            ot = stage.tile([P, V], F32, tag=f"ot{tg}{b}")
            nc.scalar.activation(out=ot, in_=po[:, :V],
                                 func=mybir.ActivationFunctionType.Copy,
                                 scale=r)
            nc.sync.dma_start(out=self.dram[5][b], in_=ot)


def build_nc(vls, repeat=1):
    """vls: (vl_a, vl_b) exact K-extents for the two local batches."""
    from contextlib import ExitStack
    nc = bacc.Bacc("TRN2", target_bir_lowering=False, debug=False,
                   num_devices=N_CORES, enable_partition_id=False)
    queries_d = nc.dram_tensor("queries", [B_LOC, Q, H], F32, kind="ExternalInput").ap()
    keys_d = nc.dram_tensor("keys", [B_LOC, K, H], F32, kind="ExternalInput").ap()
    values_d = nc.dram_tensor("values", [B_LOC, K, V], F32, kind="ExternalInput").ap()
    wq_d = nc.dram_tensor("Wq", [H, H], F32, kind="ExternalInput").ap()
    wk_d = nc.dram_tensor("Wk", [H, H], F32, kind="ExternalInput").ap()
    wv_d = nc.dram_tensor("Wv", [H], F32, kind="ExternalInput").ap()
    out_d = nc.dram_tensor("out", [B_LOC, Q, V], F32, kind="ExternalOutput").ap()
    dram = (queries_d, keys_d, values_d, wq_d, wk_d, out_d)

    with tile.TileContext(nc) as tc, ExitStack() as ctx:
        const = ctx.enter_context(tc.tile_pool(name="const", bufs=1))
        identf = const.tile([P, P], F16)
        make_identity(nc, identf)
        identf32 = const.tile([P, P], F32)
        make_identity(nc, identf32)
        wv_sb = const.tile([P, HT], F32)
        nc.sync.dma_start(out=wv_sb, in_=wv_d.rearrange("(t p) -> p t", p=P))
        bias_z = const.tile([P, 1], F32)
        nc.gpsimd.memset(bias_z, 0.0)
        bias_ph = const.tile([P, 1], F32)
        nc.gpsimd.memset(bias_ph, math.pi / 2)
        consts = {"identf": identf, "identf32": identf32, "wv_sb": wv_sb,
                  "bias_z": bias_z, "bias_ph": bias_ph}

        pools = {
            "stage": ctx.enter_context(tc.tile_pool(name="stage", bufs=1)),
            "xpool": ctx.enter_context(tc.tile_pool(name="xpool", bufs=1)),
            "bpool": ctx.enter_context(tc.tile_pool(name="bpool", bufs=3)),
            "btpool": ctx.enter_context(tc.tile_pool(name="btpool", bufs=2)),
            "sqpool": ctx.enter_context(tc.tile_pool(name="sq", bufs=2)),
            "ps_pq": ctx.enter_context(
                tc.tile_pool(name="pspq", bufs=1, space="PSUM")),
            "ps_pk": ctx.enter_context(
                tc.tile_pool(name="pspk", bufs=2, space="PSUM")),
            "ps_sc": ctx.enter_context(
                tc.tile_pool(name="pssc", bufs=1, space="PSUM")),
            "ps_tr32": ctx.enter_context(
                tc.tile_pool(name="pst32", bufs=1, space="PSUM")),
            "ps_tr16": ctx.enter_context(
                tc.tile_pool(name="pst16", bufs=1, space="PSUM")),
            "ps_po": ctx.enter_context(
                tc.tile_pool(name="pspo", bufs=1, space="PSUM")),
        }
        em = Emitter(nc, tc, pools, consts, dram)
        phA = Phase("A", vls)
        phB = Phase("B", vls)

        if repeat <= 6:
            phs = [phA, phB]
            for i in range(0, repeat, 2):
                pair = phs if repeat - i >= 2 else phs[:1]
                for p in pair:
                    em.loads(p)
                for p in pair:
                    em.part1(p)
                for p in pair:
                    em.part2(p)
        else:
            n2 = repeat // 2
            rem = repeat - 2 * n2
            with tc.For_i(0, n2, 1):
                em.loads(phA)
                em.loads(phB)
                em.part1(phA)
                em.part1(phB)
                em.part2(phA)
                em.part2(phB)
            if rem:
                em.loads(phA)
                em.part1(phA)
                em.part2(phA)
    nc.compile()
    return nc


def _make_single_core_runner(nc, device):
    """jit the program once for one device; reusable across calls."""
    bass2jax.install_neuronx_cc_hook()
    assert nc.partition_id_tensor is None
    in_names, out_names, out_avals, zero_shapes = [], [], [], []
    for alloc in nc.m.functions[0].allocations:
        if not isinstance(alloc, mybir.MemoryLocationSet):
            continue
        name = alloc.memorylocations[0].name
        if alloc.kind == "ExternalInput":
            in_names.append(name)
        elif alloc.kind == "ExternalOutput":
            shape = tuple(alloc.tensor_shape)
            npdt = np.dtype(mybir.dt.np(alloc.dtype))
            out_names.append(name)
            out_avals.append(jax.core.ShapedArray(shape, npdt))
            zero_shapes.append((shape, npdt))
    n_params = len(in_names)
    n_outs = len(out_avals)
    in_names_all = list(in_names) + list(out_names)

    def _body(*args):
        outs = bass2jax._bass_exec_p.bind(
            *args,
            out_avals=tuple(out_avals),
            in_names=tuple(in_names_all),
            out_names=tuple(out_names),
            lowering_input_output_aliases=(),
            sim_require_finite=True,
            sim_require_nnan=True,
            nc=nc,
        )
        return tuple(outs)

    fn = jax.jit(_body, donate_argnums=tuple(range(n_params, n_params + n_outs)),
                 keep_unused=True)
    sharding = SingleDeviceSharding(device)
    dev_in_cache = {}

    def launch(in_map):
        key = id(in_map)
        if key not in dev_in_cache:
            dev_in_cache.clear()
            dev_in_cache[key] = [
                jax.device_put(np.asarray(in_map[name]), sharding)
                for name in in_names
            ]
        args = list(dev_in_cache[key])
        args += [jax.device_put(np.zeros(s, d), sharding) for (s, d) in zero_shapes]
        outs = fn(*args)
        return dict(zip(out_names, outs))

    return launch


_NCS = {}       # (vls, repeat) -> compiled nc
_LAUNCH = {}    # (vls, repeat, core) -> launch fn


def _get_launch(vls, repeat, core):
    key = (vls, repeat, core)
    if key not in _LAUNCH:
        nckey = (vls, repeat)
        if nckey not in _NCS:
            _NCS[nckey] = build_nc(vls, repeat)
        _LAUNCH[key] = _make_single_core_runner(_NCS[nckey], jax.devices()[core])
    return _LAUNCH[key]


def plan_assignment(valid_lens):
    """Pair batches to balance per-core work; returns (perm, vls_per_core).

    perm[2c], perm[2c+1] are the global batch indices handled by core c.
    """
    vle = [min(K, int(v)) for v in valid_lens]
    order = sorted(range(B), key=lambda i: -vle[i])
    perm, vls_per_core = [], []
    for c in range(N_CORES):
        a, b_ = order[c], order[2 * N_CORES - 1 - c]
        perm += [a, b_]
        vls_per_core.append((vle[a], vle[b_]))
    return perm, vls_per_core


def run_cores(in_maps, vls_per_core, repeat=1, fetch=True):
    """Launch all 8 per-core programs concurrently; returns per-core out dicts."""
    outs = [
        _get_launch(vls_per_core[c], repeat, c)(in_maps[c]) for c in range(N_CORES)
    ]
    jax.block_until_ready([list(o.values()) for o in outs])
    if not fetch:
        return None
    return [{k: np.asarray(v) for k, v in o.items()} for o in outs]


def make_in_maps(queries, keys, values, Wq, Wk, Wv, valid_lens, perm):
    queries = np.asarray(queries, np.float32)
    keys = np.asarray(keys, np.float32)
    values = np.asarray(values, np.float32)
    Wq = np.asarray(Wq, np.float32)
    Wk = np.asarray(Wk, np.float32)
    Wv = np.asarray(Wv, np.float32)
    in_maps = []
    for c in range(N_CORES):
        ix = [perm[2 * c], perm[2 * c + 1]]
        in_maps.append({
            "queries": queries[ix], "keys": keys[ix], "values": values[ix],
            "Wq": Wq, "Wk": Wk, "Wv": Wv,
        })
    return in_maps


def kernel(queries, keys, values, Wq, Wk, Wv, valid_lens):
    perm, vls_per_core = plan_assignment(valid_lens)
    in_maps = make_in_maps(queries, keys, values, Wq, Wk, Wv, valid_lens, perm)
    res = run_cores(in_maps, vls_per_core)
    out = np.empty((B, Q, V), np.float32)
    for c in range(N_CORES):
        out[perm[2 * c]] = res[c]["out"][0]
        out[perm[2 * c + 1]] = res[c]["out"][1]
    return out


# revision 13
# speedup vs baseline: 1.2330x; 1.0595x over previous
"""AdditiveAttention Trainium2 kernel (Bass/Tile), 8-core data-parallel.

Math (per batch b):
    q = queries @ Wq.T              [Q, H]
    k = keys @ Wk.T                 [K, H]
    scores[q,k] = sum_h Wv[h] * tanh(q[q,h] + k[k,h])
    attn = softmax(mask(scores))    positions >= valid_len -> 0 weight
    out = attn @ values             [Q, V]

Algorithm: tanh(x) on |x|<=11.45 approximated by a half-integer sine series
    tanh(x) ~= sum_m c_m sin(w_m x),   w_m = (m - 1/2) * pi / L
Each term factorizes over the q/k split
    sin(w(q+k)) = sin(wq)cos(wk) + cos(wq)sin(wk)
so scores become PE contractions over (h, m, sin/cos).

v4 implementation notes:
  - The repeat loop is a 3-stage For_i_pipelined pipeline (loads ->
    prep -> main): while DVE runs iteration i's Chebyshev ladder, ACT/PE
    run iteration i+1's transposes/projections/seeds and SP issues
    iteration i+2's DMA loads.  Intermediate tiles are double-buffered by
    the pipeline allocator.
  - Weight preparation (Wq/Wk load, transpose, fp16 convert, Wv) is
    loop-invariant and hoisted out of the repeat loop.
  - The ladder (s_{m+1} = C s_m - s_{m-1}, C = 2cos(th) = 2-4sin^2(th/2))
    is column-split: DVE ladders [q-cols | head k-cols] while GpSimd
    (Pool) independently ladders the tail k-columns in a separate tile;
    the split point is solved per (vl_a, vl_b) to balance the engines.
  - fp32 -> fp16 conversion is folded into the PSUM->SBUF eviction copies
    of the (fp32) PE transposes, and into Pool copies for values.
  - ACT Sin seeds run straight from the projection PSUM; Square (same
    trig table set) forms C; Exp runs straight from the score PSUM.
  - Masking is exact and free: columns k >= valid_len are never computed
    (programs are specialized per (vl_a, vl_b) pair).
"""

import math
import numpy as np

import jax
from jax.sharding import SingleDeviceSharding

import concourse.bass as bass
import concourse.mybir as mybir
import concourse.tile as tile
from concourse import bacc, bass2jax
from concourse.masks import make_identity

B, Q, K, H, V = 16, 128, 512, 256, 256
N_CORES = 8
B_LOC = B // N_CORES  # 2 batches per core
P = 128
HT = H // P   # 2 h-tiles
DT = H // P   # 2 d-tiles (projection contraction)
F32 = mybir.dt.float32
F16 = mybir.dt.float16

# tanh(x) ~= sum_m C_SIN[m] * sin((m+0.5)*pi/L * x); minimax fit |x|<=11.45
L_FIT = 11.6
TH = math.pi / L_FIT
C_SIN = [1.26359946, 0.39701109, 0.21221088, 0.12884517, 0.08154461,
         0.05276574, 0.03415187, 0.02241366, 0.01448774, 0.01666857]
M_HARM = len(C_SIN)


def _ceil_even(x):
    return (x + 1) // 2 * 2


def _solve_pool_cols(ktot):
    """Columns of the concatenated k-range laddered on Pool (tail)."""
    best, bestd = 0, None
    for pk in range(0, max(ktot - 1, 1), 2):
        dve = 6700 + 16 * (60 + 2.083 * (2 * Q + ktot - pk))
        pool = 3500 + 18 * (3.33 * pk + 95)
        d = abs(dve - pool)
        if bestd is None or d < bestd:
            best, bestd = pk, d
    return best


class Layout:
    """Column layout for one (vl_a, vl_b) configuration."""

    def __init__(self, vls):
        self.vls = list(vls)
        self.vle = [_ceil_even(v) for v in vls]
        self.kbn = [(v + P - 1) // P for v in vls]
        self.ktot = self.vle[0] + self.vle[1]
        self.pk = _solve_pool_cols(self.ktot)
        self.km = self.ktot - self.pk        # k-cols in the main tile
        self.NPm = 2 * Q + self.km           # main stream width
        self.koff = [0, self.vle[0]]

    def _ranges(self, lo, hi):
        out = []
        if lo < self.km:
            out.append((0, min(hi, self.km) - lo, lo))
        if hi > self.km:
            lo2 = max(lo, self.km)
            out.append((1, hi - lo2, lo2))
        return out

    def k_ranges(self, b):
        """[(tile_sel, count, k_axis_start)] covering batch b's vl cols."""
        return self._ranges(self.koff[b], self.koff[b] + self.vls[b])

    def k_ranges_e(self, b):
        """Same but covering the even-padded extent (for seeds)."""
        return self._ranges(self.koff[b], self.koff[b] + self.vle[b])


def build_nc(vls, repeat=1):
    """vls: (vl_a, vl_b) exact K-extents for the two local batches."""
    from contextlib import ExitStack
    nc = bacc.Bacc("TRN2", target_bir_lowering=False, debug=False,
                   num_devices=N_CORES, enable_partition_id=False)
    queries_d = nc.dram_tensor("queries", [B_LOC, Q, H], F32, kind="ExternalInput").ap()
    keys_d = nc.dram_tensor("keys", [B_LOC, K, H], F32, kind="ExternalInput").ap()
    values_d = nc.dram_tensor("values", [B_LOC, K, V], F32, kind="ExternalInput").ap()
    wq_d = nc.dram_tensor("Wq", [H, H], F32, kind="ExternalInput").ap()
    wk_d = nc.dram_tensor("Wk", [H, H], F32, kind="ExternalInput").ap()
    wv_d = nc.dram_tensor("Wv", [H], F32, kind="ExternalInput").ap()
    out_d = nc.dram_tensor("out", [B_LOC, Q, V], F32, kind="ExternalOutput").ap()

    lay = Layout(vls)
    vle, kbn, NPm, pk = lay.vle, lay.kbn, lay.NPm, lay.pk

    with tile.TileContext(nc) as tc, ExitStack() as ctx:
        const = ctx.enter_context(tc.tile_pool(name="const", bufs=1))
        identf = const.tile([P, P], F16)
        make_identity(nc, identf)
        identf32 = const.tile([P, P], F32)
        make_identity(nc, identf32)
        wv_sb = const.tile([P, HT], F32)
        nc.sync.dma_start(out=wv_sb, in_=wv_d.rearrange("(t p) -> p t", p=P))
        bias_z = const.tile([P, 1], F32)
        nc.gpsimd.memset(bias_z, 0.0)
        bias_ph = const.tile([P, 1], F32)
        nc.gpsimd.memset(bias_ph, math.pi / 2)

        stage = ctx.enter_context(tc.tile_pool(name="stage", bufs=1))
        bpool = ctx.enter_context(tc.tile_pool(name="bpool", bufs=3))
        btpool = ctx.enter_context(tc.tile_pool(name="btpool", bufs=2))
        bppool = ctx.enter_context(tc.tile_pool(name="bppool", bufs=3))
        btppool = ctx.enter_context(tc.tile_pool(name="btppool", bufs=2))
        sqpool = ctx.enter_context(tc.tile_pool(name="sq", bufs=2))
        ps_pq = ctx.enter_context(tc.tile_pool(name="pspq", bufs=1, space="PSUM"))
        ps_pk = ctx.enter_context(tc.tile_pool(name="pspk", bufs=2, space="PSUM"))
        ps_sc = ctx.enter_context(tc.tile_pool(name="pssc", bufs=1, space="PSUM"))
        ps_tr32 = ctx.enter_context(tc.tile_pool(name="pst32", bufs=1, space="PSUM"))
        ps_tr16 = ctx.enter_context(tc.tile_pool(name="pst16", bufs=1, space="PSUM"))
        ps_po = ctx.enter_context(tc.tile_pool(name="pspo", bufs=1, space="PSUM"))

        # loop-invariant weight prep: load fp32, PE-transpose, fp16 evict
        wq32 = const.tile([P, HT, H], F32)
        nc.sync.dma_start(out=wq32, in_=wq_d.rearrange("(t p) d -> p t d", p=P))
        wk32 = const.tile([P, HT, H], F32)
        nc.sync.dma_start(out=wk32, in_=wk_d.rearrange("(t p) d -> p t d", p=P))
        wqT = const.tile([P, DT, H], F16)  # [d_in, dt, h]
        wkT = const.tile([P, DT, H], F16)
        trt = ps_tr32.tile([P, 4, P], F32, tag="tr32", name="tr32w")
        j = 0
        for (w32, w_T) in ((wq32, wqT), (wk32, wkT)):
            for ht in range(HT):
                for dt in range(DT):
                    if j == 4:
                        trt = ps_tr32.tile([P, 4, P], F32, tag="tr32",
                                           name="tr32w")
                        j = 0
                    nc.tensor.transpose(trt[:, j, :],
                                        w32[:, ht, dt * P:(dt + 1) * P],
                                        identf32)
                    nc.scalar.copy(out=w_T[:, dt, ht * P:(ht + 1) * P],
                                   in_=trt[:, j, :])
                    j += 1

        # ---------------- stage 0: DMA loads --------------------------------
        def s_load(pipe, iv):
            q32 = pipe.intermediate_tile([P, B_LOC, H], F32, name="q32")
            nc.sync.dma_start(out=q32,
                              in_=queries_d.rearrange("b q d -> q b d"))
            outs = [q32]
            for b in range(B_LOC):
                kb = kbn[b]
                k32 = pipe.intermediate_tile([P, kb, H], F32, name=f"k32_{b}")
                nc.sync.dma_start(
                    out=k32,
                    in_=keys_d[b, :kb * P].rearrange("(kt p) d -> p kt d", p=P))
                outs.append(k32)
                v32 = pipe.intermediate_tile([P, kb, V], F32, name=f"v32_{b}")
                nc.sync.dma_start(
                    out=v32,
                    in_=values_d[b, :kb * P].rearrange("(kt p) d -> p kt d",
                                                       p=P))
                outs.append(v32)
            return tuple(outs)

        # ---------------- stage 1: transposes, projections, seeds, setup ----
        def s_prep(pipe, iv, loaded):
            (q32, k32a, v32a, k32b, v32b) = loaded
            k32 = [k32a, k32b]
            v32 = [v32a, v32b]

            tr_state = {"tile": None, "j": 4}

            def tr_slot():
                if tr_state["j"] == 4:
                    tr_state["tile"] = ps_tr32.tile([P, 4, P], F32,
                                                    tag="tr32", name="tr32")
                    tr_state["j"] = 0
                j2 = tr_state["j"]
                tr_state["j"] += 1
                return tr_state["tile"][:, j2, :]

            # values: convert on Pool, append ones column
            vo = []
            for b in range(B_LOC):
                kb = kbn[b]
                vot = pipe.intermediate_tile([P, kb, V + 1], F16,
                                             name=f"vo_{b}")
                nc.gpsimd.tensor_copy(out=vot[:, :, :V], in_=v32[b])
                nc.gpsimd.memset(vot[:, :, V:V + 1], 1.0)
                vo.append(vot)

            # fp32 PE transposes; the evict copy converts to fp16 on ACT
            qT = stage.tile([P, DT, 2 * Q], F16, tag="qT", name="qT")
            for b in range(B_LOC):
                for dt in range(DT):
                    ps = tr_slot()
                    nc.tensor.transpose(ps, q32[:, b, dt * P:(dt + 1) * P],
                                        identf32)
                    nc.scalar.copy(out=qT[:, dt, b * Q:(b + 1) * Q], in_=ps)
            kT = []
            for b in range(B_LOC):
                kb = kbn[b]
                kTb = stage.tile([P, DT, kb * P], F16, tag=f"kT{b}",
                                 name="kTb")
                for kt_i in range(kb):
                    for dt in range(DT):
                        ps = tr_slot()
                        nc.tensor.transpose(
                            ps, k32[b][:, kt_i, dt * P:(dt + 1) * P],
                            identf32)
                        nc.scalar.copy(
                            out=kTb[:, dt, kt_i * P:(kt_i + 1) * P], in_=ps)
                kT.append(kTb)

            X1m = pipe.intermediate_tile([P, HT, 2, NPm], F16, name="X1m")
            X1p = (pipe.intermediate_tile([P, HT, 2, pk], F16, name="X1p")
                   if pk else X1m)

            # projections into PSUM; ACT Sin seeds straight from PSUM
            for ht in range(HT):
                pq = ps_pq.tile([P, 2 * Q], F32, tag="pq", name="pq")
                for dt in range(DT):
                    nc.tensor.matmul(pq, wqT[:, dt, ht * P:(ht + 1) * P],
                                     qT[:, dt, :],
                                     start=(dt == 0), stop=(dt == DT - 1))
                for (side, bias) in ((0, bias_z), (1, bias_ph)):
                    nc.scalar.activation(
                        out=X1m[:, ht, side, 0:2 * Q], in_=pq,
                        func=mybir.ActivationFunctionType.Sin,
                        bias=bias, scale=TH / 2)
                for b in range(B_LOC):
                    ve = vle[b]
                    pkps = ps_pk.tile([P, 512], F32, tag="pk", name="pkps")
                    for dt in range(DT):
                        nc.tensor.matmul(pkps[:, :ve],
                                         wkT[:, dt, ht * P:(ht + 1) * P],
                                         kT[b][:, dt, :ve],
                                         start=(dt == 0), stop=(dt == DT - 1))
                    for (side, bias) in ((0, bias_z), (1, bias_ph)):
                        src_off = 0
                        for (sel, cnt, ks) in lay.k_ranges_e(b):
                            if sel == 0:
                                dst = X1m[:, ht, side,
                                          2 * Q + ks:2 * Q + ks + cnt]
                            else:
                                dst = X1p[:, ht, side,
                                          ks - lay.km:ks - lay.km + cnt]
                            nc.scalar.activation(
                                out=dst, in_=pkps[:, src_off:src_off + cnt],
                                func=mybir.ActivationFunctionType.Sin,
                                bias=bias, scale=TH / 2)
                            src_off += cnt

            # u = sin(th/2)^2 (Square shares the trig table set with Sin)
            usqm = stage.tile([P, HT, NPm], F16, tag="usqm", name="usqm")
            nc.scalar.activation(out=usqm, in_=X1m[:, :, 0, :],
                                 func=mybir.ActivationFunctionType.Square,
                                 bias=bias_z)
            if pk:
                usqp = stage.tile([P, HT, pk], F16, tag="usqp", name="usqp")
                nc.scalar.activation(out=usqp, in_=X1p[:, :, 0, :],
                                     func=mybir.ActivationFunctionType.Square,
                                     bias=bias_z)

            # DVE setup: C streams for both ladders (fast 4x ts ops)
            C2m = pipe.intermediate_tile([P, HT, 2, NPm], F16, name="C2m")
            nc.vector.tensor_scalar(out=C2m[:, :, 0, :], in0=usqm,
                                    scalar1=-4.0, scalar2=2.0,
                                    op0=mybir.AluOpType.mult,
                                    op1=mybir.AluOpType.add)
            nc.vector.tensor_copy(out=C2m[:, :, 1, :], in_=C2m[:, :, 0, :])
            Cp1m = pipe.intermediate_tile([P, HT, NPm], F16, name="Cp1m")
            nc.vector.tensor_scalar_add(Cp1m, C2m[:, :, 0, :], 1.0)
            Cm1m = pipe.intermediate_tile([P, HT, NPm], F16, name="Cm1m")
            nc.vector.tensor_scalar_add(Cm1m, C2m[:, :, 0, :], -1.0)
            if pk:
                C2p = pipe.intermediate_tile([P, HT, 2, pk], F16, name="C2p")
                nc.vector.tensor_scalar(out=C2p[:, :, 0, :], in0=usqp,
                                        scalar1=-4.0, scalar2=2.0,
                                        op0=mybir.AluOpType.mult,
                                        op1=mybir.AluOpType.add)
                nc.vector.tensor_copy(out=C2p[:, :, 1, :], in_=C2p[:, :, 0, :])
                Cp1p = pipe.intermediate_tile([P, HT, pk], F16, name="Cp1p")
                nc.vector.tensor_scalar_add(Cp1p, C2p[:, :, 0, :], 1.0)
                Cm1p = pipe.intermediate_tile([P, HT, pk], F16, name="Cm1p")
                nc.vector.tensor_scalar_add(Cm1p, C2p[:, :, 0, :], -1.0)
            else:
                C2p, Cp1p, Cm1p = C2m, Cp1m, Cm1m
            return (X1m, X1p, C2m, Cp1m, Cm1m, C2p, Cp1p, Cm1p, vo[0], vo[1])

        # ---------------- stage 2: ladders, scores, softmax, AV, out --------
        def s_main(pipe, iv, prep):
            (X1m, X1p, C2m, Cp1m, Cm1m, C2p, Cp1p, Cm1p, vo0, vo1) = prep
            vo = [vo0, vo1]

            sc_ps = [ps_sc.tile([P, 512], F32, tag=f"sc{b}", name="sc")
                     for b in range(B_LOC)]

            def score_mms(m, bm, bp):
                if m == 1:
                    sq = sqpool.tile([P, HT, 2, 2 * Q], F16, tag="sq",
                                     name="sq1")
                    for ht in range(HT):
                        nc.vector.tensor_scalar(
                            out=sq[:, ht], in0=bm[:, ht, :, 0:2 * Q],
                            scalar1=wv_sb[:, ht:ht + 1],
                            scalar2=float(C_SIN[0]),
                            op0=mybir.AluOpType.mult,
                            op1=mybir.AluOpType.mult)
                else:
                    sq = sqpool.tile([P, HT, 2, 2 * Q], F16, tag="sq",
                                     name="sqm")
                    nc.vector.tensor_scalar_mul(sq, bm[:, :, :, 0:2 * Q],
                                                float(C_SIN[m - 1]))
                for b in range(B_LOC):
                    first = (m == 1)
                    last = (m == M_HARM)
                    ranges = lay.k_ranges(b)
                    nmm = 2 * HT * len(ranges)
                    i = 0
                    for ht in range(HT):
                        for (sqside, bside) in ((0, 1), (1, 0)):
                            for (sel, cnt, ks) in ranges:
                                if sel == 0:
                                    basis = bm[:, ht, bside,
                                               2 * Q + ks:2 * Q + ks + cnt]
                                else:
                                    basis = bp[:, ht, bside,
                                               ks - lay.km:ks - lay.km + cnt]
                                d0 = ks - lay.koff[b]
                                nc.tensor.matmul(
                                    sc_ps[b][:, d0:d0 + cnt],
                                    sq[:, ht, sqside, b * Q:(b + 1) * Q],
                                    basis,
                                    start=(first and i == 0),
                                    stop=(last and i == nmm - 1))
                                i += 1

            score_mms(1, X1m, X1p)
            # fold Wv into q-cols of the seed stream; later harmonics inherit
            for ht in range(HT):
                nc.vector.tensor_scalar_mul(X1m[:, ht, :, 0:2 * Q],
                                            X1m[:, ht, :, 0:2 * Q],
                                            wv_sb[:, ht:ht + 1])
            b2m = bpool.tile([P, HT, 2, NPm], F16, tag="bm", name="b2m")
            nc.vector.tensor_tensor(b2m[:, :, 0, :], Cp1m, X1m[:, :, 0, :],
                                    mybir.AluOpType.mult)
            nc.vector.tensor_tensor(b2m[:, :, 1, :], Cm1m, X1m[:, :, 1, :],
                                    mybir.AluOpType.mult)
            b2p = None
            if pk:
                b2p = bppool.tile([P, HT, 2, pk], F16, tag="bp", name="b2p")
                nc.gpsimd.tensor_tensor(b2p[:, :, 0, :], Cp1p,
                                        X1p[:, :, 0, :], mybir.AluOpType.mult)
                nc.gpsimd.tensor_tensor(b2p[:, :, 1, :], Cm1p,
                                        X1p[:, :, 1, :], mybir.AluOpType.mult)
            score_mms(2, b2m, b2p)
            prev2m, prevm = X1m, b2m
            prev2p, prevp = X1p, b2p
            for m in range(3, M_HARM + 1):
                tm = btpool.tile([P, HT, 2, NPm], F16, tag="btm", name="btm")
                nc.vector.tensor_tensor(tm, C2m, prevm, mybir.AluOpType.mult)
                bmm = bpool.tile([P, HT, 2, NPm], F16, tag="bm", name="bmm")
                nc.vector.tensor_tensor(bmm, tm, prev2m,
                                        mybir.AluOpType.subtract)
                bmp = None
                if pk:
                    tp = btppool.tile([P, HT, 2, pk], F16, tag="btp",
                                      name="btp")
                    nc.gpsimd.tensor_tensor(tp, C2p, prevp,
                                            mybir.AluOpType.mult)
                    bmp = bppool.tile([P, HT, 2, pk], F16, tag="bp",
                                      name="bmp")
                    nc.gpsimd.tensor_tensor(bmp, tp, prev2p,
                                            mybir.AluOpType.subtract)
                score_mms(m, bmm, bmp)
                prev2m, prevm = prevm, bmm
                prev2p, prevp = prevp, bmp

            # epilogue: exp (from score PSUM), transpose, AV, normalize
            tr_state = {"tile": None, "j": 4}

            def tr_slot():
                if tr_state["j"] == 4:
                    tr_state["tile"] = ps_tr16.tile([P, 4, P], F16,
                                                    tag="tr16", name="tr16")
                    tr_state["j"] = 0
                j2 = tr_state["j"]
                tr_state["j"] += 1
                return tr_state["tile"][:, j2, :]

            for b in range(B_LOC):
                vl = lay.vls[b]
                kb = kbn[b]
                e = stage.tile([P, K], F16, tag=f"e{b}", name="e")
                nc.scalar.activation(out=e[:, :vl], in_=sc_ps[b][:, :vl],
                                     func=mybir.ActivationFunctionType.Exp,
                                     bias=bias_z)
                po = ps_po.tile([P, V + 1], F32, tag="po", name="po")
                for kt_i in range(kb):
                    cols = min(P, vl - kt_i * P)
                    ps = tr_slot()
                    nc.tensor.transpose(ps[:cols, :],
                                        e[:, kt_i * P:kt_i * P + cols],
                                        identf)
                    eT = stage.tile([P, Q], F16, tag="eT", name="eT")
                    nc.scalar.copy(out=eT[:cols, :], in_=ps[:cols, :])
                    nc.tensor.matmul(po, eT[:cols, :], vo[b][:cols, kt_i, :],
                                     start=(kt_i == 0), stop=(kt_i == kb - 1))
                r = stage.tile([P, 1], F32, tag=f"recip{b}", name="recip")
                nc.vector.reciprocal(out=r, in_=po[:, V:V + 1])
                ot = stage.tile([P, V], F32, tag=f"ot{b}", name="ot")
                nc.scalar.activation(out=ot, in_=po[:, :V],
                                     func=mybir.ActivationFunctionType.Copy,
                                     scale=r)
                nc.sync.dma_start(out=out_d[b], in_=ot)

        tc.For_i_pipelined([s_load, s_prep, s_main], 0, repeat, unroll=2)
    nc.compile()
    return nc


def _make_single_core_runner(nc, device):
    """jit the program once for one device; reusable across calls."""
    bass2jax.install_neuronx_cc_hook()
    assert nc.partition_id_tensor is None
    in_names, out_names, out_avals, zero_shapes = [], [], [], []
    for alloc in nc.m.functions[0].allocations:
        if not isinstance(alloc, mybir.MemoryLocationSet):
            continue
        name = alloc.memorylocations[0].name
        if alloc.kind == "ExternalInput":
            in_names.append(name)
        elif alloc.kind == "ExternalOutput":
            shape = tuple(alloc.tensor_shape)
            npdt = np.dtype(mybir.dt.np(alloc.dtype))
            out_names.append(name)
            out_avals.append(jax.core.ShapedArray(shape, npdt))
            zero_shapes.append((shape, npdt))
    n_params = len(in_names)
    n_outs = len(out_avals)
    in_names_all = list(in_names) + list(out_names)

    def _body(*args):
        outs = bass2jax._bass_exec_p.bind(
            *args,
            out_avals=tuple(out_avals),
            in_names=tuple(in_names_all),
            out_names=tuple(out_names),
            lowering_input_output_aliases=(),
            sim_require_finite=True,
            sim_require_nnan=True,
            nc=nc,
        )
        return tuple(outs)

    fn = jax.jit(_body, donate_argnums=tuple(range(n_params, n_params + n_outs)),
                 keep_unused=True)
    sharding = SingleDeviceSharding(device)
    dev_in_cache = {}

    def launch(in_map):
        key = id(in_map)
        if key not in dev_in_cache:
            dev_in_cache.clear()
            dev_in_cache[key] = [
                jax.device_put(np.asarray(in_map[name]), sharding)
                for name in in_names
            ]
        args = list(dev_in_cache[key])
        args += [jax.device_put(np.zeros(s, d), sharding) for (s, d) in zero_shapes]
        outs = fn(*args)
        return dict(zip(out_names, outs))

    return launch


_NCS = {}       # (vls, repeat) -> compiled nc
_LAUNCH = {}    # (vls, repeat, core) -> launch fn


def _get_launch(vls, repeat, core):
    key = (vls, repeat, core)
    if key not in _LAUNCH:
        nckey = (vls, repeat)
        if nckey not in _NCS:
            _NCS[nckey] = build_nc(vls, repeat)
        _LAUNCH[key] = _make_single_core_runner(_NCS[nckey], jax.devices()[core])
    return _LAUNCH[key]


def plan_assignment(valid_lens):
    """Pair batches to balance per-core work; returns (perm, vls_per_core).

    perm[2c], perm[2c+1] are the global batch indices handled by core c.
    """
    vle = [min(K, int(v)) for v in valid_lens]
    order = sorted(range(B), key=lambda i: -vle[i])
    perm, vls_per_core = [], []
    for c in range(N_CORES):
        a, b_ = order[c], order[2 * N_CORES - 1 - c]
        perm += [a, b_]
        vls_per_core.append((vle[a], vle[b_]))
    return perm, vls_per_core


def run_cores(in_maps, vls_per_core, repeat=1, fetch=True):
    """Launch all 8 per-core programs concurrently; returns per-core out dicts."""
    outs = [
        _get_launch(vls_per_core[c], repeat, c)(in_maps[c]) for c in range(N_CORES)
    ]
    jax.block_until_ready([list(o.values()) for o in outs])
    if not fetch:
        return None
    return [{k: np.asarray(v) for k, v in o.items()} for o in outs]


def make_in_maps(queries, keys, values, Wq, Wk, Wv, valid_lens, perm):
    queries = np.asarray(queries, np.float32)
    keys = np.asarray(keys, np.float32)
    values = np.asarray(values, np.float32)
    Wq = np.asarray(Wq, np.float32)
    Wk = np.asarray(Wk, np.float32)
    Wv = np.asarray(Wv, np.float32)
    in_maps = []
    for c in range(N_CORES):
        ix = [perm[2 * c], perm[2 * c + 1]]
        in_maps.append({
            "queries": queries[ix], "keys": keys[ix], "values": values[ix],
            "Wq": Wq, "Wk": Wk, "Wv": Wv,
        })
    return in_maps


def kernel(queries, keys, values, Wq, Wk, Wv, valid_lens):
    perm, vls_per_core = plan_assignment(valid_lens)
    in_maps = make_in_maps(queries, keys, values, Wq, Wk, Wv, valid_lens, perm)
    res = run_cores(in_maps, vls_per_core)
    out = np.empty((B, Q, V), np.float32)
    for c in range(N_CORES):
        out[perm[2 * c]] = res[c]["out"][0]
        out[perm[2 * c + 1]] = res[c]["out"][1]
    return out


# revision 17
# speedup vs baseline: 1.8342x; 1.4876x over previous
"""AdditiveAttention Trainium2 kernel (Bass/Tile), 8-core data-parallel.

Math (per batch b):
    q = queries @ Wq.T              [Q, H]
    k = keys @ Wk.T                 [K, H]
    scores[q,k] = sum_h Wv[h] * tanh(q[q,h] + k[k,h])
    attn = softmax(mask(scores))    positions >= valid_len -> 0 weight
    out = attn @ values             [Q, V]

Algorithm: tanh(x) on |x|<=12 is approximated by a sine expansion
    tanh(x) ~= sum_m c_m sin(w_m x),   w_m = (m - 1/2) * pi / L,  L = 12
(half-integer harmonics: the antiperiodic extension of tanh is smooth, so
the series converges geometrically; M=10 gives |err| ~ 1.3e-2).  Each term
factorizes over the q/k split:
    sin(w(q+k)) = sin(wq)cos(wk) + cos(wq)sin(wk)
so scores become a single PE contraction over (h, m, sin/cos):
    scores[q,k] = sum_{h,m} [cwv(m,h) sin_q][cos_k] + [cwv cos_q][sin_k]
with cwv(m,h) = c_m * Wv[h].  This moves the O(Q*K*H) elementwise work of
the reference onto the TensorEngine; per-element work is only the harmonic
basis (computed once per q-row and k-row element).

Per core (2 batches), per iteration:
  - DMA inputs; GpSimd converts to fp16; PE transposes (identity matmul);
    PE projections (fp16) into PSUM.
  - ACT seeds s1=sin(th/2), c1=cos(th/2) (th = pi*x/L, |th|<=pi so the
    Sin table range [-pi,pi] holds) straight from the projection PSUM,
    packed into one per-batch stream [h, ht, {sin,cos}, q|k] fp16.
  - DVE Chebyshev ladder: C=2cos(th)=2-4*s1^2; s_{m+1}=C*s_m - s_{m-1}
    (sin and cos sequences packed side by side -> one mult + one sub per
    step).  Per-m q-slices scaled by c_m*Wv via fused tensor_scalar (4x).
  - PE: 4 matmuls per (ht, m) accumulate scores^ [q, k] per batch.
  - ACT Exp straight from score PSUM (scores are O(3.6): no max-sub, fp16
    safe); PE transposes exp tiles; AV matmul with [values|1] fp16 gives
    output and softmax denominator; DVE reciprocal + ACT scale.
ACT instructions are emitted Sin-block-then-Exp-block so only 2 activation
table reloads (1.3us each) occur per iteration.
Masking is exact and free: columns k >= valid_len are simply never
computed (programs are specialized per (vl_a, vl_b) pair).
"""

import math
import numpy as np

import jax
from jax.sharding import SingleDeviceSharding

import concourse.bass as bass
import concourse.mybir as mybir
import concourse.tile as tile
from concourse import bacc, bass2jax
from concourse.masks import make_identity

B, Q, K, H, V = 16, 128, 512, 256, 256
N_CORES = 8
B_LOC = B // N_CORES  # 2 batches per core
P = 128
HT = H // P   # 2 h-tiles
DT = H // P   # 2 d-tiles (projection contraction)
F32 = mybir.dt.float32
F16 = mybir.dt.float16
BF16 = mybir.dt.bfloat16

# tanh(x) ~= sum_m C_SIN[m] * sin((m+0.5)*pi/L * x), fit on |x|<=11.6
L_FIT = 12.0
TH = math.pi / L_FIT
C_SIN = [1.26351633, 0.3994312, 0.21398227, 0.1323217, 0.08358727,
         0.05585199, 0.03551782, 0.02475266, 0.01529648, 0.01932612]
M_HARM = len(C_SIN)


def _emit_weights(nc, tc, ctx):
    """Loop-invariant: identity, biases, Wv, Wq/Wk transposed fp16."""
    const = ctx.enter_context(tc.tile_pool(name="const", bufs=1))
    ps_tr = ctx.enter_context(tc.tile_pool(name="ps_tr", bufs=2, space="PSUM"))
    identf = const.tile([P, P], F16)
    make_identity(nc, identf)
    wq_nat = const.tile([P, HT, H], F32)
    nc.sync.dma_start(out=wq_nat, in_=nc._wq_d.rearrange("(t p) d -> p t d", p=P))
    wk_nat = const.tile([P, HT, H], F32)
    nc.sync.dma_start(out=wk_nat, in_=nc._wk_d.rearrange("(t p) d -> p t d", p=P))
    wq16 = const.tile([P, HT, H], F16)
    nc.gpsimd.tensor_copy(out=wq16, in_=wq_nat)
    wk16 = const.tile([P, HT, H], F16)
    nc.gpsimd.tensor_copy(out=wk16, in_=wk_nat)
    tr_state = {"tile": None, "j": 4}

    def tr_slot():
        if tr_state["j"] == 4:
            tr_state["tile"] = ps_tr.tile([P, 4, P], F16, tag="ps_tr",
                                          name="trps0")
            tr_state["j"] = 0
        j = tr_state["j"]
        tr_state["j"] += 1
        return tr_state["tile"][:, j, :]

    wqT = const.tile([P, DT, H], F16)  # [d_in, dt, h]
    wkT = const.tile([P, DT, H], F16)
    for (w16, w_T) in ((wq16, wqT), (wk16, wkT)):
        for ht in range(HT):
            for dt in range(DT):
                ps = tr_slot()
                nc.tensor.transpose(ps, w16[:, ht, dt * P:(dt + 1) * P], identf)
                nc.scalar.copy(out=w_T[:, dt, ht * P:(ht + 1) * P], in_=ps)
    wv_sb = const.tile([P, HT], F32)
    nc.gpsimd.dma_start(out=wv_sb, in_=nc._wv_d.rearrange("(t p) -> p t", p=P))
    bias_z = const.tile([P, 1], F32)
    nc.gpsimd.memset(bias_z, 0.0)
    bias_ph = const.tile([P, 1], F32)
    nc.gpsimd.memset(bias_ph, math.pi / 2)
    return {"identf": identf, "wqT": wqT, "wkT": wkT, "wv_sb": wv_sb,
            "bias_z": bias_z, "bias_ph": bias_ph, "ps_tr": ps_tr}


def _emit(nc, tc, vls, queries_d, keys_d, values_d, wq_d, wk_d, wv_d,
          out_d, ctx, W):
    stage = ctx.enter_context(tc.tile_pool(name="stage", bufs=2))
    xpool = ctx.enter_context(tc.tile_pool(name="xpool", bufs=2))
    bpool = ctx.enter_context(tc.tile_pool(name="bpool", bufs=4))
    sqpool = ctx.enter_context(tc.tile_pool(name="sqpool", bufs=4))
    ps_k = ctx.enter_context(tc.tile_pool(name="ps_k", bufs=2, space="PSUM"))
    ps_sc = ctx.enter_context(tc.tile_pool(name="ps_sc", bufs=1, space="PSUM"))
    ps_tr = W["ps_tr"]
    ps_sm = ctx.enter_context(tc.tile_pool(name="ps_sm", bufs=1, space="PSUM"))

    kbn = [(v + P - 1) // P for v in vls]
    vlp = [min((v + 1) // 2 * 2, kb * P) for v, kb in zip(vls, kbn)]

    identf = W["identf"]
    wqT = W["wqT"]
    wkT = W["wkT"]
    wv_sb = W["wv_sb"]
    bias_z = W["bias_z"]
    bias_ph = W["bias_ph"]

    # transpose PSUM slots: 4 x [128,128] f16 share one 2KB bank
    tr_state = {"tile": None, "j": 4}

    def tr_slot():
        if tr_state["j"] == 4:
            tr_state["tile"] = ps_tr.tile([P, 4, P], F16, tag="ps_tr", name="trps")
            tr_state["j"] = 0
        j = tr_state["j"]
        tr_state["j"] += 1
        return tr_state["tile"][:, j, :]

    # ---------------- per-batch prologue: load, convert, transpose, project,
    # seeds.  xs[b][ht] = [h128, {sin,cos}, q | k] fp16 seed stream.
    xs = []
    vo16 = []
    for b in range(B_LOC):
        vl = vls[b]
        kb = kbn[b]
        kpad = kb * P

        q_nat = stage.tile([P, H], F32, tag=f"qnat{b}")
        nc.sync.dma_start(out=q_nat, in_=queries_d[b])
        k_nat = stage.tile([P, kb, H], F32, tag=f"knat{b}")
        nc.sync.dma_start(
            out=k_nat,
            in_=keys_d[b, :kb * P].rearrange("(kt p) d -> p kt d", p=P))
        vo_f = stage.tile([P, kb, V], F32, tag=f"vof{b}")
        nc.sync.dma_start(
            out=vo_f,
            in_=values_d[b, :kb * P].rearrange("(kt p) d -> p kt d", p=P))
        vo = stage.tile([P, kb, V + 1], F16, tag=f"vo{b}")
        nc.gpsimd.tensor_copy(out=vo[:, :, :V], in_=vo_f)
        nc.gpsimd.memset(vo[:, :, V:V + 1], 1.0)
        vo16.append(vo)

        q16 = stage.tile([P, H], F16, tag=f"q16{b}")
        nc.gpsimd.tensor_copy(out=q16, in_=q_nat)
        k16 = stage.tile([P, kb, H], F16, tag=f"k16{b}")
        nc.gpsimd.tensor_copy(out=k16, in_=k_nat)

        qTd = stage.tile([P, DT, Q], F16, tag=f"qTd{b}")  # [d_in, dt, qi]
        for dt in range(DT):
            ps = tr_slot()
            nc.tensor.transpose(ps, q16[:, dt * P:(dt + 1) * P], identf)
            nc.scalar.copy(out=qTd[:, dt, :], in_=ps)
        kTd = stage.tile([P, DT, kpad], F16, tag=f"kTd{b}")
        for kt in range(kb):
            for dt in range(DT):
                ps = tr_slot()
                nc.tensor.transpose(ps, k16[:, kt, dt * P:(dt + 1) * P], identf)
                nc.scalar.copy(out=kTd[:, dt, kt * P:(kt + 1) * P], in_=ps)

        vp = vlp[b]
        xa = xpool.tile([P, HT, 2, Q + vp], F16, tag=f"x{b}")
        for ht in range(HT):
            pq = ps_sm.tile([P, Q], F32, tag="psq")
            for dt in range(DT):
                nc.tensor.matmul(pq, wqT[:, dt, ht * P:(ht + 1) * P], qTd[:, dt, :],
                                 start=(dt == 0), stop=(dt == DT - 1))
            pk = ps_k.tile([P, K], F32, tag="psk")
            for dt in range(DT):
                nc.tensor.matmul(pk[:, :vp], wkT[:, dt, ht * P:(ht + 1) * P],
                                 kTd[:, dt, :vp], start=(dt == 0), stop=(dt == DT - 1))
            # seeds (ACT, Sin table) straight from projection PSUM
            for (side, bias) in ((0, bias_z), (1, bias_ph)):
                nc.scalar.activation(
                    out=xa[:, ht, side, 0:Q], in_=pq,
                    func=mybir.ActivationFunctionType.Sin,
                    bias=bias, scale=TH / 2)
                nc.scalar.activation(
                    out=xa[:, ht, side, Q:Q + vp], in_=pk[:, :vp],
                    func=mybir.ActivationFunctionType.Sin,
                    bias=bias, scale=TH / 2)
        xs.append(xa)

    # ---------------- Chebyshev ladder + score matmuls, per batch ----------
    sc_ps = []
    for b in range(B_LOC):
        sct = ps_sc.tile([P, K], F32, tag=f"sc{b}")
        sc_ps.append(sct)

    for b in range(B_LOC):
        vl = vls[b]
        N = Q + vlp[b]

        def score_mms(m, basis):
            if m == 1:
                # m=1 q-slices are pre-fold: scale by Wv*c_1 per ht
                sq = sqpool.tile([P, HT, 2, Q], F16, tag=f"sq{b}", name="sq1")
                for ht in range(HT):
                    nc.vector.tensor_scalar(
                        out=sq[:, ht], in0=basis[:, ht, :, 0:Q],
                        scalar1=wv_sb[:, ht:ht + 1], scalar2=float(C_SIN[0]),
                        op0=mybir.AluOpType.mult, op1=mybir.AluOpType.mult)
            else:
                # q-slices already carry Wv (folded into the seed stream):
                # one constant scale covers both ht halves and both sides
                sq = sqpool.tile([P, HT, 2, Q], F16, tag=f"sq{b}", name="sqm")
                nc.vector.tensor_scalar_mul(sq, basis[:, :, :, 0:Q],
                                            float(C_SIN[m - 1]))
            for ht in range(HT):
                first = (ht == 0 and m == 1)
                last = (ht == HT - 1 and m == M_HARM)
                # sin_q * cos_k  +  cos_q * sin_k
                nc.tensor.matmul(sc_ps[b][:, :vl], sq[:, ht, 0, :],
                                 basis[:, ht, 1, Q:Q + vl], start=first, stop=False)
                nc.tensor.matmul(sc_ps[b][:, :vl], sq[:, ht, 1, :],
                                 basis[:, ht, 0, Q:Q + vl], start=False, stop=last)

        xa = xs[b]
        s1 = xa[:, :, 0, :]
        c1 = xa[:, :, 1, :]
        t0 = stage.tile([P, HT, N], F16, tag=f"t0{b}")
        nc.vector.tensor_tensor(t0, s1, s1, mybir.AluOpType.mult)
        Cf = stage.tile([P, HT, N], F16, tag=f"Cf{b}")
        nc.vector.tensor_scalar(out=Cf, in0=t0, scalar1=-4.0, scalar2=2.0,
                                op0=mybir.AluOpType.mult, op1=mybir.AluOpType.add)
        score_mms(1, xa)
        # fold Wv into the q-part of the seed stream (in-place, after all
        # unscaled reads above -- DVE executes in emission order); every
        # later harmonic inherits the scaling through the linear recurrence
        for ht in range(HT):
            nc.vector.tensor_scalar_mul(xa[:, ht, :, 0:Q], xa[:, ht, :, 0:Q],
                                        wv_sb[:, ht:ht + 1])
        Cp1 = stage.tile([P, HT, N], F16, tag=f"Cp{b}")
        nc.vector.tensor_scalar_add(Cp1, Cf, 1.0)
        Cm1 = stage.tile([P, HT, N], F16, tag=f"Cm{b}")
        nc.vector.tensor_scalar_add(Cm1, Cf, -1.0)
        b2 = bpool.tile([P, HT, 2, N], F16, tag=f"b{b}")
        nc.vector.tensor_tensor(b2[:, :, 0, :], Cp1, s1, mybir.AluOpType.mult)
        nc.vector.tensor_tensor(b2[:, :, 1, :], Cm1, c1, mybir.AluOpType.mult)
        score_mms(2, b2)
        prev2, prev = xa, b2
        Cb = Cf[:, :, None, :].to_broadcast((P, HT, 2, N))
        for m in range(3, M_HARM + 1):
            t = bpool.tile([P, HT, 2, N], F16, tag=f"bt{b}")
            nc.vector.tensor_tensor(t, Cb, prev, mybir.AluOpType.mult)
            bm = bpool.tile([P, HT, 2, N], F16, tag=f"b{b}")
            nc.vector.tensor_tensor(bm, t, prev2, mybir.AluOpType.subtract)
            score_mms(m, bm)
            prev2, prev = prev, bm

    # ---------------- exp (Exp table), AV, normalize, per batch -------------
    for b in range(B_LOC):
        vl = vls[b]
        kb = kbn[b]
        e = stage.tile([P, K], F16, tag=f"e{b}")
        nc.scalar.activation(out=e[:, :vl], in_=sc_ps[b][:, :vl],
                             func=mybir.ActivationFunctionType.Exp, bias=bias_z)
        po = ps_sm.tile([P, V + 1], F32, tag="po")
        for kt in range(kb):
            cols = min(P, vl - kt * P)
            ps = tr_slot()
            nc.tensor.transpose(ps[:cols, :], e[:, kt * P:kt * P + cols], identf)
            eT = stage.tile([P, Q], F16, tag="eT")
            nc.scalar.copy(out=eT[:cols, :], in_=ps[:cols, :])
            nc.tensor.matmul(po, eT[:cols, :], vo16[b][:cols, kt, :],
                             start=(kt == 0), stop=(kt == kb - 1))
        r = stage.tile([P, 1], F32, tag=f"recip{b}")
        nc.vector.reciprocal(out=r, in_=po[:, V:V + 1])
        ot = stage.tile([P, V], F32, tag=f"ot{b}")
        nc.scalar.activation(out=ot, in_=po[:, :V],
                             func=mybir.ActivationFunctionType.Copy, scale=r)
        nc.sync.dma_start(out=out_d[b], in_=ot)


def build_nc(vls, repeat=1):
    """vls: (vl_a, vl_b) exact K-extents for the two local batches."""
    from contextlib import ExitStack
    nc = bacc.Bacc("TRN2", target_bir_lowering=False, debug=False,
                   num_devices=N_CORES, enable_partition_id=False)
    queries_d = nc.dram_tensor("queries", [B_LOC, Q, H], F32, kind="ExternalInput").ap()
    keys_d = nc.dram_tensor("keys", [B_LOC, K, H], F32, kind="ExternalInput").ap()
    values_d = nc.dram_tensor("values", [B_LOC, K, V], F32, kind="ExternalInput").ap()
    wq_d = nc.dram_tensor("Wq", [H, H], F32, kind="ExternalInput").ap()
    wk_d = nc.dram_tensor("Wk", [H, H], F32, kind="ExternalInput").ap()
    wv_d = nc.dram_tensor("Wv", [H], F32, kind="ExternalInput").ap()
    out_d = nc.dram_tensor("out", [B_LOC, Q, V], F32, kind="ExternalOutput").ap()

    nc._wq_d, nc._wk_d, nc._wv_d = wq_d, wk_d, wv_d
    with tile.TileContext(nc) as tc, ExitStack() as ctx:
        W = _emit_weights(nc, tc, ctx)
        args = (nc, tc, vls, queries_d, keys_d, values_d, wq_d, wk_d, wv_d,
                out_d, ctx, W)
        if repeat == 1:
            _emit(*args)
        else:
            with tc.For_i(0, repeat, 1):
                _emit(*args)
    nc.compile()
    return nc


def _make_single_core_runner(nc, device):
    """jit the program once for one device; reusable across calls."""
    bass2jax.install_neuronx_cc_hook()
    assert nc.partition_id_tensor is None
    in_names, out_names, out_avals, zero_shapes = [], [], [], []
    for alloc in nc.m.functions[0].allocations:
        if not isinstance(alloc, mybir.MemoryLocationSet):
            continue
        name = alloc.memorylocations[0].name
        if alloc.kind == "ExternalInput":
            in_names.append(name)
        elif alloc.kind == "ExternalOutput":
            shape = tuple(alloc.tensor_shape)
            npdt = np.dtype(mybir.dt.np(alloc.dtype))
            out_names.append(name)
            out_avals.append(jax.core.ShapedArray(shape, npdt))
            zero_shapes.append((shape, npdt))
    n_params = len(in_names)
    n_outs = len(out_avals)
    in_names_all = list(in_names) + list(out_names)

    def _body(*args):
        outs = bass2jax._bass_exec_p.bind(
            *args,
            out_avals=tuple(out_avals),
            in_names=tuple(in_names_all),
            out_names=tuple(out_names),
            lowering_input_output_aliases=(),
            sim_require_finite=True,
            sim_require_nnan=True,
            nc=nc,
        )
        return tuple(outs)

    fn = jax.jit(_body, donate_argnums=tuple(range(n_params, n_params + n_outs)),
                 keep_unused=True)
    sharding = SingleDeviceSharding(device)
    dev_in_cache = {}

    def launch(in_map):
        key = id(in_map)
        if key not in dev_in_cache:
            dev_in_cache.clear()
            dev_in_cache[key] = [
                jax.device_put(np.asarray(in_map[name]), sharding)
                for name in in_names
            ]
        args = list(dev_in_cache[key])
        args += [jax.device_put(np.zeros(s, d), sharding) for (s, d) in zero_shapes]
        outs = fn(*args)
        return dict(zip(out_names, outs))

    return launch


_NCS = {}       # (vls, repeat) -> compiled nc
_LAUNCH = {}    # (vls, repeat, core) -> launch fn


def _get_launch(vls, repeat, core):
    key = (vls, repeat, core)
    if key not in _LAUNCH:
        nckey = (vls, repeat)
        if nckey not in _NCS:
            _NCS[nckey] = build_nc(vls, repeat)
        _LAUNCH[key] = _make_single_core_runner(_NCS[nckey], jax.devices()[core])
    return _LAUNCH[key]


def plan_assignment(valid_lens):
    """Pair batches to balance per-core work; returns (perm, vls_per_core).

    perm[2c], perm[2c+1] are the global batch indices handled by core c.
    """
    vle = [min(K, int(v)) for v in valid_lens]
    order = sorted(range(B), key=lambda i: -vle[i])
    perm, vls_per_core = [], []
    for c in range(N_CORES):
        a, b_ = order[c], order[2 * N_CORES - 1 - c]
        perm += [a, b_]
        vls_per_core.append((vle[a], vle[b_]))
    return perm, vls_per_core


def run_cores(in_maps, vls_per_core, repeat=1, fetch=True):
    """Launch all 8 per-core programs concurrently; returns per-core out dicts."""
    outs = [
        _get_launch(vls_per_core[c], repeat, c)(in_maps[c]) for c in range(N_CORES)
    ]
    jax.block_until_ready([list(o.values()) for o in outs])
    if not fetch:
        return None
    return [{k: np.asarray(v) for k, v in o.items()} for o in outs]


def make_in_maps(queries, keys, values, Wq, Wk, Wv, valid_lens, perm):
    queries = np.asarray(queries, np.float32)
    keys = np.asarray(keys, np.float32)
    values = np.asarray(values, np.float32)
    Wq = np.asarray(Wq, np.float32)
    Wk = np.asarray(Wk, np.float32)
    Wv = np.asarray(Wv, np.float32)
    in_maps = []
    for c in range(N_CORES):
        ix = [perm[2 * c], perm[2 * c + 1]]
        in_maps.append({
            "queries": queries[ix], "keys": keys[ix], "values": values[ix],
            "Wq": Wq, "Wk": Wk, "Wv": Wv,
        })
    return in_maps


def kernel(queries, keys, values, Wq, Wk, Wv, valid_lens):
    perm, vls_per_core = plan_assignment(valid_lens)
    in_maps = make_in_maps(queries, keys, values, Wq, Wk, Wv, valid_lens, perm)
    res = run_cores(in_maps, vls_per_core)
    out = np.empty((B, Q, V), np.float32)
    for c in range(N_CORES):
        out[perm[2 * c]] = res[c]["out"][0]
        out[perm[2 * c + 1]] = res[c]["out"][1]
    return out

